# revision 1
# baseline (speedup 1.0000x reference)
"""Causal multi-head attention kernel for TRN2 (8 NeuronCores).

Problem: x[64,64,512] -> flatten N=4096 tokens; Q/K/V proj (512->512, 8 heads x 64);
causal softmax(QK^T/8) @ V; output proj. edge_index unused; temporal_mask == tril.

Sharding (zero-collective): 4 head-groups (2 heads each) x 2 seq-groups.
Each core: its head-group's K^T/V for ALL tokens, Q^T for its zigzag-balanced
2048 rows (8 chunks of 256, padded-uniform slot extents E_s = 2(s+1) blocks),
transposed-scores flash attention (kv on partitions, q on free dim; softmax
sum via ones-matmul; division deferred past nothing -- applied pre O-proj),
partial O-projection over its 128 channels. Host sums 4 head-group partials
per seq-group, un-zigzags rows.
"""

import numpy as np

# ---------------- constants ----------------
N = 4096
C = 512
H = 8
D = 64
P = 128
NSLOT = 8          # q-chunks of 256 rows per core
CHUNK = 256
CORES = list(range(8))

# zigzag chunk assignment per seq-group: slot s holds chunk CHUNKS[sg][s],
# slot extent = 2*(s+1) blocks of 256 kv rows (>= chunk's causal need).
CHUNKS = [
    [0, 3, 4, 7, 8, 11, 12, 15],
    [1, 2, 5, 6, 9, 10, 13, 14],
]

_cache = {}


def _build_program():
    import concourse.bass as bass
    import concourse.mybir as mybir
    import concourse.tile as tile
    from concourse import bacc

    f32 = mybir.dt.float32
    bf16 = mybir.dt.bfloat16
    Exp = mybir.ActivationFunctionType.Exp

    nc = bacc.Bacc(None, target_bir_lowering=False)

    # inputs (per-core data, identical shapes -> one SPMD program)
    xt = nc.declare_dram_parameter("xt", [P, 4, N], bf16, isOutput=False)       # x^T  [c-tiles]
    xtq = nc.declare_dram_parameter("xtq", [P, 4, 2048], bf16, isOutput=False)  # own q-cols of x^T
    wq = nc.declare_dram_parameter("wq", [P, 4, P], bf16, isOutput=False)       # Wq^T slice (c-tiles, 128 out-ch)
    wk = nc.declare_dram_parameter("wk", [P, 4, P], bf16, isOutput=False)
    wv = nc.declare_dram_parameter("wv", [P, 4, P], bf16, isOutput=False)
    wo = nc.declare_dram_parameter("wo", [P, C], bf16, isOutput=False)          # Wo^T rows for this hg
    msk = nc.declare_dram_parameter("msk", [P, NSLOT, 2, 2, CHUNK], bf16, isOutput=False)
    ident = nc.declare_dram_parameter("ident", [P, P], bf16, isOutput=False)
    ones16 = nc.declare_dram_parameter("ones16", [P, 1], bf16, isOutput=False)
    ones32 = nc.declare_dram_parameter("ones32", [1, D], f32, isOutput=False)
    outT = nc.declare_dram_parameter("outT", [C, 2048], f32, isOutput=True)     # final^T partial

    with tile.TileContext(nc) as tc:
        with tc.tile_pool(name="const", bufs=1) as cp:
            xt_sb = cp.tile([P, 4, N], bf16)
            xtq_sb = cp.tile([P, 4, 2048], bf16)
            wq_sb = cp.tile([P, 4, P], bf16)
            wk_sb = cp.tile([P, 4, P], bf16)
            wv_sb = cp.tile([P, 4, P], bf16)
            wo_sb = cp.tile([P, C], bf16)
            msk_sb = cp.tile([P, NSLOT, 2, 2, CHUNK], bf16)
            id_sb = cp.tile([P, P], bf16)
            o16_sb = cp.tile([P, 1], bf16)
            o32_sb = cp.tile([1, D], f32)
            kt_sb = cp.tile([P, N], bf16)       # K^T (2 heads stacked on partitions)
            qt_sb = cp.tile([P, 2048], bf16)    # Q^T/8
            vt_sb = cp.tile([P, N], bf16)       # V^T
            v2_sb = cp.tile([P, 32, P], bf16)   # V   [m-within-tile, mt, d2]
            acc_sb = cp.tile([P, 2, 4, 2 * CHUNK], bf16)  # P-sum accum (h, sub-pass, slot-pair cols)
            ao_sb = cp.tile([P, 2048], bf16)    # normalized attn-out^T
            rc_sb = cp.tile([2, NSLOT, CHUNK], f32)  # reciprocal denominators

            for ap, dram in [(xt_sb, xt), (xtq_sb, xtq), (wq_sb, wq), (wk_sb, wk),
                             (wv_sb, wv), (wo_sb, wo), (msk_sb, msk), (id_sb, ident),
                             (o16_sb, ones16), (o32_sb, ones32)]:
                nc.sync.dma_start(out=ap[:], in_=dram[:])

            nc.gpsimd.memset(acc_sb[:], 0.0)

            # ---------- phase 1: projections ----------
            with tc.tile_pool(name="pp", bufs=2, space="PSUM") as pp:
                for n in range(8):  # K^T
                    ps = pp.tile([P, 512], f32, tag="proj")
                    for ct in range(4):
                        nc.tensor.matmul(ps[:], wk_sb[:, ct], xt_sb[:, ct, n * 512:(n + 1) * 512],
                                         start=(ct == 0), stop=(ct == 3))
                    nc.scalar.copy(out=kt_sb[:, n * 512:(n + 1) * 512], in_=ps[:])
                for n in range(4):  # Q^T (scaled by 1/sqrt(D))
                    ps = pp.tile([P, 512], f32, tag="proj")
                    for ct in range(4):
                        nc.tensor.matmul(ps[:], wq_sb[:, ct], xtq_sb[:, ct, n * 512:(n + 1) * 512],
                                         start=(ct == 0), stop=(ct == 3))
                    nc.scalar.mul(qt_sb[:, n * 512:(n + 1) * 512], ps[:], 1.0 / 8.0)
                for n in range(8):  # V^T
                    ps = pp.tile([P, 512], f32, tag="proj")
                    for ct in range(4):
                        nc.tensor.matmul(ps[:], wv_sb[:, ct], xt_sb[:, ct, n * 512:(n + 1) * 512],
                                         start=(ct == 0), stop=(ct == 3))
                    nc.scalar.copy(out=vt_sb[:, n * 512:(n + 1) * 512], in_=ps[:])
                for mt in range(32):  # V^T -> V (PE transpose)
                    tp = pp.tile([P, P], bf16, tag="tp")
                    nc.tensor.transpose(tp[:], vt_sb[:, mt * P:(mt + 1) * P], id_sb[:])
                    nc.scalar.copy(out=v2_sb[:, mt, :], in_=tp[:])

            # ---------- phase 2: attention (two passes over slot halves) ----------
            with (tc.tile_pool(name="stp", bufs=2, space="PSUM") as stp,
                  tc.tile_pool(name="pvp", bufs=1, space="PSUM") as pvp,
                  tc.tile_pool(name="dnp", bufs=1, space="PSUM") as dnp,
                  tc.tile_pool(name="bcp", bufs=1, space="PSUM") as bcp,
                  tc.tile_pool(name="ppool", bufs=6) as ppool,
                  tc.tile_pool(name="rcp", bufs=2) as rcp):
                for pas in range(4):
                    # sub-pass over slots (s_even, s_odd); pv = exactly one PSUM bank
                    pv = pvp.tile([P, 2, CHUNK], f32, tag="pv")
                    s_even, s_odd = 2 * pas, 2 * pas + 1
                    npairs = 2 * (2 * pas + 2)
                    for k in range(npairs):
                        # while both slots are active, one N=512 matmul covers their
                        # adjacent q-columns; afterwards only the odd slot remains.
                        both = k < 2 * (2 * pas + 1)
                        W = 2 * CHUNK if both else CHUNK
                        qoff = (s_even if both else s_odd) * CHUNK
                        for t, mt in enumerate((2 * k, 2 * k + 1)):
                            # concurrent row-packed heads in different PSUM banks
                            # (same-bank concurrent PE writes fault the device).
                            st = stp.tile([P, 2, 2 * CHUNK], f32, tag="st")
                            for h in range(2):
                                nc.tensor.matmul(
                                    st[:, h, 0:W],
                                    kt_sb[h * D:(h + 1) * D, mt * P:(mt + 1) * P],
                                    qt_sb[h * D:(h + 1) * D, qoff:qoff + W],
                                    start=True, stop=True)
                            pt = ppool.tile([P, 2, 2 * CHUNK], bf16, tag="pt")
                            nc.scalar.activation(pt[:, :, 0:W], st[:, :, 0:W], Exp)
                            # last-2-block masking (even slot's region is always in
                            # the merged cols 0:CHUNK; odd slot's is single-active)
                            if k in (2 * s_even, 2 * s_even + 1):
                                j = k - 2 * s_even
                                for h in range(2):
                                    nc.vector.tensor_mul(out=pt[:, h, 0:CHUNK], in0=pt[:, h, 0:CHUNK],
                                                         in1=msk_sb[:, s_even, j, t])
                            if k in (2 * s_odd, 2 * s_odd + 1):
                                j = k - 2 * s_odd
                                for h in range(2):
                                    nc.vector.tensor_mul(out=pt[:, h, 0:CHUNK], in0=pt[:, h, 0:CHUNK],
                                                         in1=msk_sb[:, s_odd, j, t])
                            if both:
                                nc.vector.tensor_add(out=acc_sb[:, :, pas, :], in0=acc_sb[:, :, pas, :],
                                                     in1=pt[:])
                            else:
                                nc.vector.tensor_add(out=acc_sb[:, :, pas, CHUNK:2 * CHUNK],
                                                     in0=acc_sb[:, :, pas, CHUNK:2 * CHUNK],
                                                     in1=pt[:, :, 0:CHUNK])
                            for h in range(2):
                                out_ap = pv[h * D:(h + 1) * D, :, :] if both \
                                    else pv[h * D:(h + 1) * D, 1, :]
                                nc.tensor.matmul(
                                    out_ap,
                                    v2_sb[:, mt, h * D:(h + 1) * D],
                                    pt[:, h, 0:W],
                                    start=(k == 0 and t == 0),
                                    stop=(k == npairs - 1 and t == 1),
                                    skip_group_check=True)
                    # sub-pass end: normalize both finished slots at once (512 cols)
                    for h in range(2):
                        dn = dnp.tile([1, 2 * CHUNK], f32, tag="dn")
                        nc.tensor.matmul(dn[:], o16_sb[:, 0:1], acc_sb[:, h, pas, :],
                                         start=True, stop=True)
                        rc = rcp.tile([1, 2 * CHUNK], f32, tag="rc")
                        nc.vector.reciprocal(rc[:], dn[:])
                        bc = bcp.tile([D, 2 * CHUNK], f32, tag="bc")
                        nc.tensor.matmul(bc[:], o32_sb[:], rc[:], start=True, stop=True)
                        bcs = rcp.tile([D, 2 * CHUNK], f32, tag="bcs")
                        nc.vector.tensor_copy(out=bcs[:], in_=bc[:])
                        nc.vector.tensor_mul(
                            out=ao_sb[h * D:(h + 1) * D, 2 * pas * CHUNK:(2 * pas + 2) * CHUNK],
                            in0=pv[h * D:(h + 1) * D].rearrange("p a b -> p (a b)"), in1=bcs[:])

            # ---------- phase 3: partial O-projection ----------
            with (tc.tile_pool(name="op", bufs=2, space="PSUM") as op,
                  tc.tile_pool(name="ostage", bufs=3) as ost):
                for o in range(4):
                    for n in range(4):
                        ps = op.tile([P, 512], f32, tag="oproj")
                        nc.tensor.matmul(ps[:], wo_sb[:, o * P:(o + 1) * P],
                                         ao_sb[:, n * 512:(n + 1) * 512],
                                         start=True, stop=True)
                        os_t = ost.tile([P, 512], f32, tag="ostage")
                        nc.vector.tensor_copy(out=os_t[:], in_=ps[:])
                        nc.sync.dma_start(out=outT[o * P:(o + 1) * P, n * 512:(n + 1) * 512],
                                          in_=os_t[:])
    nc.finalize()
    return nc


def _part_major(a, parts=128):
    # [K, M] with K = ct*128 -> [128, ct, M]
    K, M = a.shape
    return np.ascontiguousarray(a.reshape(K // parts, parts, M).transpose(1, 0, 2))


def _build_inputs(x, Wq, Wk, Wv, Wo):
    import ml_dtypes
    bf16 = ml_dtypes.bfloat16

    xf = np.asarray(x, np.float32).reshape(N, C)
    xT = np.ascontiguousarray(xf.T)                          # [512, 4096]
    xt_h = _part_major(xT.astype(bf16))                      # [128, 4, 4096]

    # per-slot mask for the last 2 kv blocks (4 m-tiles), per seq-group
    msks = []
    for sg in range(2):
        m = np.zeros((P, NSLOT, 2, 2, CHUNK), np.float32)
        for s in range(NSLOT):
            c = CHUNKS[sg][s]
            for j in range(2):
                for i in range(2):
                    mt = 4 * s + 2 * j + i
                    kv = mt * P + np.arange(P)[:, None]
                    q = c * CHUNK + np.arange(CHUNK)[None, :]
                    m[:, s, j, i, :] = (kv <= q).astype(np.float32)
        msks.append(m.astype(bf16))

    xtqs = []
    for sg in range(2):
        cols = np.concatenate([np.arange(c * CHUNK, (c + 1) * CHUNK) for c in CHUNKS[sg]])
        xtqs.append(_part_major(np.ascontiguousarray(xT[:, cols]).astype(bf16)))

    in_maps = []
    for core in CORES:
        hg, sg = core // 2, core % 2
        sl = slice(hg * P, (hg + 1) * P)
        in_maps.append({
            "xt": xt_h,
            "xtq": xtqs[sg],
            "wq": _part_major(np.ascontiguousarray(Wq.T[:, sl]).astype(bf16)),
            "wk": _part_major(np.ascontiguousarray(Wk.T[:, sl]).astype(bf16)),
            "wv": _part_major(np.ascontiguousarray(Wv.T[:, sl]).astype(bf16)),
            "wo": np.ascontiguousarray(Wo.T[sl, :]).astype(bf16),
            "msk": msks[sg],
            "ident": np.eye(P, dtype=np.float32).astype(bf16),
            "ones16": np.ones((P, 1), np.float32).astype(bf16),
            "ones32": np.ones((1, D), np.float32),
        })
    return in_maps


def kernel(x, edge_index, temporal_mask, Wq, bq, Wk, bk, Wv, bv, Wo, bo):
    from concourse.bass_utils import run_bass_kernel_spmd

    if "nc" not in _cache:
        _cache["nc"] = _build_program()
    nc = _cache["nc"]

    in_maps = _build_inputs(x, Wq, Wk, Wv, Wo)
    res = run_bass_kernel_spmd(nc, in_maps, CORES).results

    out = np.zeros((N, C), np.float32)
    for sg in range(2):
        tot = np.zeros((C, 2048), np.float32)
        for hg in range(4):
            tot += np.asarray(res[hg * 2 + sg]["outT"], np.float32)
        for s in range(NSLOT):
            c = CHUNKS[sg][s]
            out[c * CHUNK:(c + 1) * CHUNK, :] = tot[:, s * CHUNK:(s + 1) * CHUNK].T
    out += np.asarray(bo, np.float32)[None, :]
    return out.astype(np.float32)

